# revision 1
# baseline (speedup 1.0000x reference)
"""ImgBEVGeneration kernel for 8 Trainium2 NeuronCores.

Strategy (per sharding hint): the 96 independent (view, image-row) groups of
the outer-product + DepthAggregation conv net are sharded 12-per-core across
the 8 NeuronCores; each core scatter-adds its points into a private
(16384, 80) BEV accumulator with segment_sum, and the per-core BEV maps are
summed with an all-reduce (lax.psum).  Voxel-index geometry depends only on
the tiny 4x4 camera matrices and is computed on host.
"""

import numpy as np

# ---- module constants ----
D_START, D_END, D_STEP = 2.0, 58.0, 0.5
OGF_H, OGF_W = 256, 704
FH, FW = 16, 44
C = 80
D = 112
VOXEL_SIZE = np.array([0.8, 0.8, 8.0], np.float32)
VOXEL_COORD = np.array([-51.2 + 0.4, -51.2 + 0.4, -5.0 + 4.0], np.float32)
NX, NY, NZ = 128, 128, 1
BN_EPS = 1e-5
N_CORES = 8
B, N = 1, 6
NCELL = NY * NX


def _frustum_np():
    d = np.arange(D_START, D_END, D_STEP, dtype=np.float32)
    xs = np.linspace(0.0, OGF_W - 1.0, FW).astype(np.float32)
    ys = np.linspace(0.0, OGF_H - 1.0, FH).astype(np.float32)
    dc = np.broadcast_to(d[:, None, None], (D, FH, FW))
    xc = np.broadcast_to(xs[None, None, :], (D, FH, FW))
    yc = np.broadcast_to(ys[None, :, None], (D, FH, FW))
    return np.stack([xc, yc, dc, np.ones_like(dc)], -1)  # (D, fH, fW, 4)


def _cells_np(sensor2ego_mats, intrin_mats, ida_mats, bda_mats):
    """Per-point BEV cell index, (N, D, fH, fW) int32; invalid -> NCELL."""
    frustum = _frustum_np()
    ida_inv = np.linalg.inv(np.asarray(ida_mats, np.float32))
    pts = np.einsum('bnij,dhwj->bndhwi', ida_inv, frustum).astype(np.float32)
    pts = np.concatenate([pts[..., :2] * pts[..., 2:3], pts[..., 2:]], -1)
    combine = np.asarray(sensor2ego_mats, np.float32) @ np.linalg.inv(
        np.asarray(intrin_mats, np.float32))
    pts = np.einsum('bnij,bndhwj->bndhwi', combine, pts).astype(np.float32)
    pts = np.einsum('bij,bndhwj->bndhwi', np.asarray(bda_mats, np.float32),
                    pts).astype(np.float32)
    geom = pts[..., :3]
    gi = ((geom - (VOXEL_COORD - VOXEL_SIZE / 2.0)) / VOXEL_SIZE).astype(np.int32)
    gx, gy, gz = gi[..., 0], gi[..., 1], gi[..., 2]
    valid = ((gx >= 0) & (gx < NX) & (gy >= 0) & (gy < NY)
             & (gz >= 0) & (gz < NZ))
    cell = gy * NX + gx
    cell = np.where(valid, cell, NCELL).astype(np.int32)
    return cell[0]  # (N, D, fH, fW)


_CACHE = {}


def _get_pmapped():
    if 'fn' in _CACHE:
        return _CACHE['fn']
    import jax
    import jax.numpy as jnp

    def conv3x3(x, w, b=None):
        y = jax.lax.conv_general_dilated(
            x, w, (1, 1), 'SAME', dimension_numbers=('NCHW', 'OIHW', 'NCHW'))
        return y if b is None else y + b[None, :, None, None]

    def bn(x, p):
        g, b_, m, v = p[0], p[1], p[2], p[3]
        s = g * jax.lax.rsqrt(v + BN_EPS)
        return x * s[None, :, None, None] + (b_ - m * s)[None, :, None, None]

    def shard_fn(img_g, depth_g, cell_g, w_red, bn_red, w_c1, bn_c1, w_c2,
                 bn_c2, w_out, b_out):
        # img_g: (G, C, FW)   depth_g: (G, D, FW)   cell_g: (G, D, FW) int32
        # x: (G, C, W, D) ;  x[g,c,w,d] = img_g[g,c,w] * depth_g[g,d,w]
        x = img_g[:, :, :, None] * jnp.transpose(depth_g, (0, 2, 1))[:, None, :, :]
        x = jax.nn.relu(bn(conv3x3(x, w_red), bn_red))
        y = jax.nn.relu(bn(conv3x3(x, w_c1), bn_c1))
        y = jax.nn.relu(bn(conv3x3(y, w_c2), bn_c2))
        y = conv3x3(y + x, w_out, b_out)              # (G, C, W, D)
        feats = jnp.transpose(y, (0, 3, 2, 1))        # (G, D, W, C)
        feats = feats.reshape(-1, C)
        cells = cell_g.transpose(0, 1, 2).reshape(-1)  # (G*D*FW,)
        pooled = jax.ops.segment_sum(feats, cells, num_segments=NCELL + 1)
        pooled = pooled[:NCELL]                        # drop invalid bucket
        return jax.lax.psum(pooled, axis_name='i')     # (NCELL, C)

    fn = jax.pmap(
        shard_fn, axis_name='i',
        in_axes=(0, 0, 0, None, None, None, None, None, None, None, None))
    _CACHE['fn'] = fn
    return fn


def kernel(img_feats, depth_feats, sensor2ego_mats, intrin_mats, ida_mats,
           bda_mats, w_red, bn_red, w_c1, bn_c1, w_c2, bn_c2, w_out, b_out):
    img_feats = np.asarray(img_feats, np.float32)      # (6, 80, 16, 44)
    depth_feats = np.asarray(depth_feats, np.float32)  # (6, 112, 16, 44)

    cells = _cells_np(sensor2ego_mats, intrin_mats, ida_mats, bda_mats)

    # 96 (n, h) groups -> 8 cores x 12 groups
    img_g = img_feats.transpose(0, 2, 1, 3).reshape(N * FH, C, FW)
    depth_g = depth_feats.transpose(0, 2, 1, 3).reshape(N * FH, D, FW)
    cell_g = cells.transpose(0, 2, 1, 3).reshape(N * FH, D, FW)

    G = N * FH // N_CORES
    img_s = img_g.reshape(N_CORES, G, C, FW)
    depth_s = depth_g.reshape(N_CORES, G, D, FW)
    cell_s = cell_g.reshape(N_CORES, G, D, FW)

    fn = _get_pmapped()
    pooled = fn(img_s, depth_s, cell_s,
                np.asarray(w_red, np.float32), np.asarray(bn_red, np.float32),
                np.asarray(w_c1, np.float32), np.asarray(bn_c1, np.float32),
                np.asarray(w_c2, np.float32), np.asarray(bn_c2, np.float32),
                np.asarray(w_out, np.float32), np.asarray(b_out, np.float32))
    pooled = np.asarray(pooled[0])                     # (NCELL, C)
    out = pooled.reshape(NY, NX, C).transpose(2, 0, 1)[None]  # (1, C, NY, NX)
    return np.ascontiguousarray(out.astype(np.float32))
